# revision 1
# baseline (speedup 1.0000x reference)
"""Causal multi-head attention (B=2, N=2048, D=1024, H=16, Dh=64) on 8 trn2 cores.

Sharding: core c = (batch b = c//4, head-quadrant g = c%4) -> 4 heads of one
batch per core.  Each core:
  - projects Q^T/K^T (per head-pair, [dh,128 x n]) and V ([n x dh]) with fp32r
    matmuls from a host-pretransposed x^T,
  - runs causal flash-style attention in the transposed orientation
    (s_T[j,q] = K^T.T @ Q^T per 128-ctx x 512-q chunk, exp on ScalarE -> bf16,
    P.V + row-sum matmuls accumulated in PSUM),
  - normalizes and applies its slice of the output projection.
Host sums the 4 partial projections per batch.
"""

import numpy as np
import ml_dtypes

B, N, D, H, Dh = 2, 2048, 1024, 16, 64
DC = D // 128          # 8 contraction chunks
NB = N // 128          # 16 ctx blocks
NS = N // 512          # 4 q strips
N_CORES = 8
SCALE = float(Dh) ** -0.5

_COMPILED = None
TRACE = False
LAST_EXEC_NS = None
LAST_RESULTS = None


def _build():
    import concourse.bass as bass
    import concourse.tile as tile
    from concourse import bacc, mybir

    f32 = mybir.dt.float32
    f32r = mybir.dt.float32r
    bf = mybir.dt.bfloat16
    EXP = mybir.ActivationFunctionType.Exp

    nc = bacc.Bacc("TRN2", target_bir_lowering=False, debug=False,
                   enable_asserts=False, num_devices=N_CORES)

    xT = nc.dram_tensor("xT", [D, N], f32, kind="ExternalInput")
    wq = nc.dram_tensor("wq", [D, 256], f32, kind="ExternalInput")
    wk = nc.dram_tensor("wk", [D, 256], f32, kind="ExternalInput")
    wv = nc.dram_tensor("wv", [D, 256], f32, kind="ExternalInput")
    wo = nc.dram_tensor("wo", [256, D], f32, kind="ExternalInput")
    bo = nc.dram_tensor("bo", [D], f32, kind="ExternalInput")
    mask = nc.dram_tensor("mask", [128, 128], mybir.dt.uint8, kind="ExternalInput")
    y = nc.dram_tensor("y", [N, D], f32, kind="ExternalOutput")

    with tile.TileContext(nc) as tc:
        from contextlib import ExitStack
        with ExitStack() as ctx:
            const = ctx.enter_context(tc.tile_pool(name="const", bufs=1))
            work = ctx.enter_context(tc.tile_pool(name="work", bufs=3))
            epool = ctx.enter_context(tc.tile_pool(name="epool", bufs=5))
            pssT = ctx.enter_context(
                tc.tile_pool(name="pssT", bufs=3, space=bass.MemorySpace.PSUM))
            pspv = ctx.enter_context(
                tc.tile_pool(name="pspv", bufs=1, space=bass.MemorySpace.PSUM))

            # ---------------- loads ----------------
            # small tensors first so their DMAs are not starved by the
            # chained xT chunk stream
            wq_sb = const.tile([128, DC, 256], f32r)
            nc.sync.dma_start(
                wq_sb[:], wq.ap().rearrange("(c p) n -> p c n", p=128).bitcast(f32r))
            wk_sb = const.tile([128, DC, 256], f32r)
            nc.sync.dma_start(
                wk_sb[:], wk.ap().rearrange("(c p) n -> p c n", p=128).bitcast(f32r))
            wv_sb = const.tile([128, DC, 256], f32r)
            nc.sync.dma_start(
                wv_sb[:], wv.ap().rearrange("(c p) n -> p c n", p=128).bitcast(f32r))
            wo_sb = const.tile([128, 2, D], f32r)
            nc.sync.dma_start(
                wo_sb[:], wo.ap().rearrange("(c p) n -> p c n", p=128).bitcast(f32r))
            mask_sb = const.tile([128, 128], mybir.dt.uint8)
            nc.sync.dma_start(mask_sb[:], mask.ap())
            neg_sb = const.tile([128, 128], f32)
            nc.vector.memset(neg_sb[:], -1.0e5)
            bo_sb = const.tile([1, D], f32)
            nc.sync.dma_start(bo_sb[:], bo.ap().rearrange("(a n) -> a n", a=1))
            Bb = const.tile([128, D], f32)
            nc.gpsimd.partition_broadcast(Bb[:], bo_sb[0:1, :])

            xT_sb = const.tile([128, DC, N], f32r)
            for d in range(DC):
                nc.sync.dma_start(
                    xT_sb[:, d, :],
                    xT.ap()[128 * d:128 * d + 128, :].bitcast(f32r))

            # ---------------- QKV projections / attention, pipelined ----------------
            def proj_qk(p):
                mats = []
                for w_sb, nm in ((wq_sb, "q"), (wk_sb, "k")):
                    dst = const.tile([128, N], f32r, tag=f"{nm}T{p}", name=f"{nm}T{p}")
                    for half in (0, 1):
                        pq = pssT.tile([128, 1024], f32, tag="sT", name=f"pq{nm}{p}{half}")
                        for ns in (0, 1):
                            osl = slice(512 * ns, 512 * ns + 512)
                            nsl = slice(1024 * half + 512 * ns,
                                        1024 * half + 512 * ns + 512)
                            for d in range(DC):
                                nc.tensor.matmul(
                                    pq[:, osl],
                                    w_sb[:, d, 128 * p:128 * p + 128],
                                    xT_sb[:, d, nsl],
                                    start=(d == 0), stop=(d == DC - 1))
                        nc.vector.tensor_copy(
                            dst[:, 1024 * half:1024 * half + 1024], pq[:])
                    mats.append(dst)
                return mats

            QT0, KT0 = proj_qk(0)
            QT = [QT0, None]
            KT = [KT0, None]

            vsb = const.tile([128, NB, 4, Dh + 1], bf)
            nc.vector.memset(vsb[:, :, :, Dh:Dh + 1], 1.0)

            def proj_v():
                for nb in range(NB):
                    pvp = pspv.tile([128, 256], f32, tag="pvA", name=f"pvp{nb}")
                    for d in range(DC):
                        nc.tensor.matmul(
                            pvp[:], xT_sb[:, d, 128 * nb:128 * nb + 128],
                            wv_sb[:, d, :], start=(d == 0), stop=(d == DC - 1))
                    nc.vector.tensor_copy(
                        vsb[:, nb, :, 0:Dh],
                        pvp[:].rearrange("p (h d) -> p h d", h=4))

            onorm = [const.tile([128, N], f32r, tag="onorm0", name="onorm0"),
                     const.tile([128, N], f32r, tag="onorm1", name="onorm1")]

            def out_proj_strip(s):
                for qb in range(4 * s, 4 * s + 4):
                    qsl = slice(128 * qb, 128 * qb + 128)
                    yp = pssT.tile([128, 1024], f32, tag="sT", name=f"yp{qb}")
                    for nst in (0, 1):
                        osl = slice(512 * nst, 512 * nst + 512)
                        for p in (0, 1):
                            nc.tensor.matmul(yp[:, osl], onorm[p][:, qsl],
                                             wo_sb[:, p, osl],
                                             start=(p == 0), stop=(p == 1))
                    ysb = work.tile([128, D], f32, tag="ysb", name=f"ysb{qb}")
                    nc.vector.tensor_add(ysb[:], yp[:], Bb[:])
                    nc.sync.dma_start(y.ap()[qsl, :], ysb[:])

            def attn_strip(p, s):
                on = onorm[p]
                qsl0 = 512 * s
                nch = 4 * (s + 1)
                pvA = pspv.tile([65, 512], f32, tag="pvA", name=f"pvA{p}{s}")
                pvB = pspv.tile([65, 512], f32, tag="pvB", name=f"pvB{p}{s}")
                batches = []
                for j0 in range(0, 4 * s, 2):
                    batches.append([(j0, 512, 0), (j0 + 1, 512, 512)])
                batches.append([(4 * s, 512, 0), (4 * s + 1, 384, 512)])
                batches.append([(4 * s + 2, 256, 0), (4 * s + 3, 128, 256)])
                for bi, batch in enumerate(batches):
                    tot = batch[-1][2] + batch[-1][1]
                    sTa = pssT.tile([128, 1024], f32, tag="sT", name=f"sTa{p}{s}{bi}")
                    sTb = pssT.tile([128, 1024], f32, tag="sT", name=f"sTb{p}{s}{bi}")
                    for (j, w, ofs) in batch:
                        off = 512 - w
                        jsl = slice(128 * j, 128 * j + 128)
                        qs = slice(qsl0 + off, qsl0 + 512)
                        nc.tensor.matmul(sTa[:, ofs:ofs + w],
                                         KT[p][0:64, jsl], QT[p][0:64, qs],
                                         start=True, stop=True)
                        nc.tensor.matmul(sTb[:, ofs:ofs + w],
                                         KT[p][64:128, jsl], QT[p][64:128, qs],
                                         start=True, stop=True)
                        if j >= 4 * s:  # diagonal chunk: mask logits in psum
                            nc.vector.copy_predicated(
                                sTa[:, ofs:ofs + 128], mask_sb[:], neg_sb[:])
                            nc.vector.copy_predicated(
                                sTb[:, ofs:ofs + 128], mask_sb[:], neg_sb[:])
                    ea = epool.tile([128, 1024], bf, tag="e", name=f"ea{p}{s}{bi}")
                    eb = epool.tile([128, 1024], bf, tag="e", name=f"eb{p}{s}{bi}")
                    nc.scalar.activation(ea[:, 0:tot], sTa[:, 0:tot], EXP,
                                         scale=SCALE)
                    nc.scalar.activation(eb[:, 0:tot], sTb[:, 0:tot], EXP,
                                         scale=SCALE)
                    for (j, w, ofs) in batch:
                        off = 512 - w
                        first = (j == 0)
                        last = (j == nch - 1)
                        nc.tensor.matmul(pvA[:, off:512],
                                         vsb[:, j, 2 * p + 0, :],
                                         ea[:, ofs:ofs + w],
                                         start=first, stop=last)
                        nc.tensor.matmul(pvB[:, off:512],
                                         vsb[:, j, 2 * p + 1, :],
                                         eb[:, ofs:ofs + w],
                                         start=first, stop=last)
                # normalize: o = pv[0:64] * (1 / pv[64]) per head
                scrA = work.tile([1, 512], f32, tag="scrA", name=f"scrA{p}{s}")
                scrB = work.tile([1, 512], f32, tag="scrB", name=f"scrB{p}{s}")
                nc.vector.reciprocal(scrA[:], pvA[64:65, :])
                nc.vector.reciprocal(scrB[:], pvB[64:65, :])
                Ra = work.tile([64, 512], f32, tag="Ra", name=f"Ra{p}{s}")
                Rb = work.tile([64, 512], f32, tag="Rb", name=f"Rb{p}{s}")
                nc.gpsimd.partition_broadcast(Ra[:], scrA[:])
                nc.gpsimd.partition_broadcast(Rb[:], scrB[:])
                qs = slice(qsl0, qsl0 + 512)
                nc.vector.tensor_mul(on[0:64, qs], pvA[0:64, :], Ra[:])
                nc.vector.tensor_mul(on[64:128, qs], pvB[0:64, :], Rb[:])

            # V projection must fully precede pair-0 attention (it shares the
            # pvA PSUM slot); pair-1 QKV is emitted after pair-0 attention so
            # its matmuls gap-fill the PE while ScalarE works through the exps
            proj_v()
            for s in range(NS):
                attn_strip(0, s)
            QT[1], KT[1] = proj_qk(1)
            for s in range(NS):
                attn_strip(1, s)
                out_proj_strip(s)

    nc.compile()
    return nc


def _get_compiled():
    global _COMPILED
    if _COMPILED is None:
        _COMPILED = _build()
    return _COMPILED


def kernel(x, w_qkv, w_out, b_out):
    global LAST_EXEC_NS, LAST_RESULTS
    from concourse.bass_utils import run_bass_kernel_spmd

    x = np.ascontiguousarray(np.asarray(x, dtype=np.float32))
    w_qkv = np.asarray(w_qkv, dtype=np.float32)
    w_out = np.asarray(w_out, dtype=np.float32)
    b_out = np.asarray(b_out, dtype=np.float32)

    mask_np = np.tril(np.ones((128, 128), dtype=np.uint8), -1)

    nc = _get_compiled()
    in_maps = []
    for c in range(N_CORES):
        b, g = divmod(c, 4)
        hs = [4 * g + i for i in range(4)]
        cols = np.concatenate([np.arange(64 * h, 64 * h + 64) for h in hs])
        in_maps.append({
            "xT": np.ascontiguousarray(x[b].T),
            "wq": np.ascontiguousarray(w_qkv[:, cols]),
            "wk": np.ascontiguousarray(w_qkv[:, D + cols]),
            "wv": np.ascontiguousarray(w_qkv[:, 2 * D + cols]),
            "wo": np.ascontiguousarray(w_out[cols, :]),
            "bo": b_out if g == 0 else np.zeros_like(b_out),
            "mask": mask_np,
        })
    res = run_bass_kernel_spmd(nc, in_maps, core_ids=list(range(N_CORES)),
                               trace=TRACE)
    LAST_EXEC_NS = res.exec_time_ns
    LAST_RESULTS = res
    ys = [res.results[c]["y"] for c in range(N_CORES)]
    out = np.stack([ys[0] + ys[1] + ys[2] + ys[3],
                    ys[4] + ys[5] + ys[6] + ys[7]])
    return out.astype(np.float32)

